# revision 14
# baseline (speedup 1.0000x reference)
"""Causal multi-head attention with RoPE on 8 Trainium2 NeuronCores.

Sharding: core c -> batch b = c // 2, head-group g = c % 2 (8 heads each).
Each core computes q/k/v projections for its 512 output dims, RoPE, causal
attention for its 8 heads, and a partial O-projection. Host sums the two
partial outputs per batch and transposes back.

Device layout notes (v3 — bf16 + resident x + warm PE):
  - x arrives bf16 and stays resident in SBUF ([128, 2048] x 8 k-tiles)
    for the whole kernel: V projection (lhsT slices) and Q/K projections
    (rhs) read it directly — x is DMA'd exactly once.
  - ~20 dummy matmuls at kernel start chew through the initial DMA wait
    so the PE HAM clock-gate is already released (2.4 GHz) when the real
    matmuls begin.
  - All matmul operands are bf16 (PSUM accumulation stays fp32).
  - q/k are kept transposed [d, s] per head-pair tile [128, 2048]
    (head 2p on partitions 0..63, head 2p+1 on 64..127).
  - RoPE: q' = q * cos + swap(q) * sin_signed, where swap is an
    adjacent-partition-pair permutation done with a 128x128 permutation
    matmul; cos (bf16) multiplies the bf16 projection copy at DVE 2x
    rate, sin (f32) multiplies the PSUM swap result.
  - Scores are computed transposed (keys on partitions) so softmax
    needs no DVE reductions: exp() goes straight from PSUM through the
    scalar engine to bf16, the denominator comes from a ones-column
    appended to V, and causal masking is a post-exp affine_select fill
    restricted to the one [128,128] block (both heads in one call) that
    actually straddles the diagonal.
  - v is stored naturally [s, d] with per-head interleaved ones columns
    ([128, 8*65] bf16 tiles) so PV lhsT slices are contiguous.
"""

import numpy as np
import ml_dtypes

import concourse.bass as bass
import concourse.tile as tile
from concourse import bacc, mybir
from concourse.bass_utils import run_bass_kernel_spmd

F32 = mybir.dt.float32
BF16 = mybir.dt.bfloat16
MULT = mybir.AluOpType.mult
IS_GE = mybir.AluOpType.is_ge
EXP = mybir.ActivationFunctionType.Exp

P = 128          # partitions
S = 2048         # sequence length
D = 1024         # model dim
DK = 64          # head dim
HPC = 8          # heads per core
NPAIR = 4        # head pairs per core
KT = 8           # 128-row k-tiles of the contraction dim (D)
CH = 512         # i-chunk width
NCH = S // CH    # 4 i-chunks
NJT = S // P     # 16 j-tiles
NDUMMY = 42      # PE-warming matmuls at kernel start

_CACHED_NC = None
LAST_RESULTS = None


def build_nc():
    nc = bacc.Bacc("TRN2", target_bir_lowering=False, debug=False)

    xT = nc.dram_tensor("xT", [D, S], BF16, kind="ExternalInput").ap()
    wq = nc.dram_tensor("wq", [D, 512], BF16, kind="ExternalInput").ap()
    wk = nc.dram_tensor("wk", [D, 512], BF16, kind="ExternalInput").ap()
    wv = nc.dram_tensor("wv", [D, 512], BF16, kind="ExternalInput").ap()
    wo = nc.dram_tensor("wo", [512, D], BF16, kind="ExternalInput").ap()
    cosn = nc.dram_tensor("cosn", [P, S], BF16, kind="ExternalInput").ap()
    sins = nc.dram_tensor("sins", [P, S], F32, kind="ExternalInput").ap()
    psw = nc.dram_tensor("psw", [P, P], BF16, kind="ExternalInput").ap()
    out = nc.dram_tensor("out", [D, S], BF16, kind="ExternalOutput").ap()

    xT3 = xT.rearrange("(kt p) s -> p kt s", p=P)
    wq3 = wq.rearrange("(kt p) o -> p kt o", p=P)
    wk3 = wk.rearrange("(kt p) o -> p kt o", p=P)
    wv3 = wv.rearrange("(kt p) o -> p kt o", p=P)
    wo3 = wo.rearrange("(pt p) o -> p pt o", p=P)

    with tile.TileContext(nc) as tc:
        with (
            tc.tile_pool(name="persist", bufs=1) as persist,
            tc.tile_pool(name="work", bufs=2) as work,
            tc.tile_pool(name="tmp", bufs=2) as tmp,
            tc.tile_pool(name="expp", bufs=4) as expp,
        ):
            cos_sb = persist.tile([P, S], BF16, tag="cos")
            sin_sb = persist.tile([P, S], F32, tag="sin")
            psw_sb = persist.tile([P, P], BF16, tag="psw")
            xkt = [persist.tile([P, S], BF16, name=f"xkt{kt}", tag=f"xkt{kt}")
                   for kt in range(KT)]
            v_sb = [persist.tile([P, HPC * 65], BF16, name=f"v{jt}", tag=f"v{jt}")
                    for jt in range(NJT)]
            dmy = persist.tile([P, 512], BF16, tag="dmy")
            nc.vector.memset(dmy[:], 0.0)
            # 0/1 causal mask for the [128,128] diagonal block (both heads):
            # mask2[ch, :, i] = 1 if i >= ch else 0
            mask2 = persist.tile([P, 2, P], BF16, tag="mask2")
            nc.vector.memset(mask2[:], 1.0)
            nc.gpsimd.affine_select(
                out=mask2[:], in_=mask2[:], compare_op=IS_GE, fill=0.0,
                base=0, channel_multiplier=-1, pattern=[[0, 2], [1, P]])
            warm = persist.tile([1, 8], F32, tag="warm")
            nc.vector.memset(warm[:], 1.0)
            # touch Exp early so the ~2.7us ACT table load overlaps DMAs
            nc.scalar.activation(warm[:], warm[:], EXP, scale=0.0)
            att_sb = [persist.tile([P, S], BF16, name=f"att{p}", tag=f"att{p}")
                      for p in range(NPAIR)]
            # ones columns of v (strided memset; projection fills the rest)
            for jt in range(NJT):
                v3m = v_sb[jt][:].rearrange("p (h e) -> p h e", e=65)
                nc.vector.memset(v3m[:, :, DK:65], 1.0)

            def p2_prefetch(pair):
                st = {}
                st["q"] = work.tile([P, S], BF16, tag="qpair", name=f"q{pair}")
                st["k"] = work.tile([P, S], BF16, tag="kpair", name=f"k{pair}")
                st["wq"] = work.tile([P, KT, P], BF16, tag="wqp", name=f"wq{pair}", bufs=1)
                st["wk"] = work.tile([P, KT, P], BF16, tag="wkp", name=f"wk{pair}", bufs=1)
                osl = slice(pair * P, (pair + 1) * P)
                nc.sync.dma_start(st["wq"][:], wq3[:, :, osl])
                nc.sync.dma_start(st["wk"][:], wk3[:, :, osl])
                return st

            # ---- single PSUM pool for all phases ----
            pp_ctx = tc.tile_pool(name="pp23", bufs=1, space="PSUM")
            pp = pp_ctx.__enter__()
            p1w_ctx = tc.tile_pool(name="p1w", bufs=1)
            p1w = p1w_ctx.__enter__()

            # PE-warming dummies: no data deps, run during the DMA wait
            for i in range(NDUMMY):
                dps = pp.tile([P, 512], F32, tag="ps2", bufs=2)
                nc.tensor.matmul(dps[:], dmy[:, 0:P], dmy[:],
                                 start=True, stop=True)
            wv_sb = p1w.tile([P, KT, 512], BF16, tag="wv")
            nc.sync.dma_start(wv_sb[:, 0:2, :], wv3[:, 0:2, :])
            for kt in range(KT):
                nc.sync.dma_start(xkt[kt][:, 0:512], xT3[:, kt, 0:512])
            nc.sync.dma_start(wv_sb[:, 2:KT, :], wv3[:, 2:KT, :])
            nc.sync.dma_start(psw_sb[:], psw)
            st0 = p2_prefetch(0)
            for kt in range(KT):
                nc.sync.dma_start(xkt[kt][:, 512:1024], xT3[:, kt, 512:1024])
            nc.sync.dma_start(cos_sb[:], cosn)
            nc.sync.dma_start(sin_sb[:], sins)
            for blk in range(2, 4):
                csl = slice(blk * 512, blk * 512 + 512)
                for kt in range(KT):
                    nc.sync.dma_start(xkt[kt][:, csl], xT3[:, kt, csl])

            def p1_vproj(st):
                ps = pp.tile([P, 512], F32, tag="ps2", bufs=2)
                for kt in range(KT):
                    nc.tensor.matmul(
                        ps[:],
                        xkt[kt][:, st * P:(st + 1) * P],
                        wv_sb[:, kt, :],
                        start=(kt == 0),
                        stop=(kt == KT - 1),
                    )
                v3 = v_sb[st][:].rearrange("p (h e) -> p h e", e=65)
                nc.scalar.copy(
                    out=v3[:, :, 0:DK],
                    in_=ps[:].rearrange("p (h d) -> p h d", d=DK),
                )

            def p2_proj(st, c, which, act_copy=False):
                # one tensor (q or k): 8-matmul projection burst + RoPE
                ssl = slice(c * CH, (c + 1) * CH)
                w_t = st["wq"] if which == "q" else st["wk"]
                dst = st["q"] if which == "q" else st["k"]
                ps2 = pp.tile([P, CH], F32, tag="ps2", bufs=2)
                for kt in range(KT):
                    nc.tensor.matmul(
                        ps2[:], w_t[:, kt, :], xkt[kt][:, ssl],
                        start=(kt == 0), stop=(kt == KT - 1))
                raw = tmp.tile([P, CH], BF16, tag="raw")
                if act_copy:
                    nc.scalar.copy(out=raw[:], in_=ps2[:])
                else:
                    nc.vector.tensor_copy(out=raw[:], in_=ps2[:])
                # dst = raw * cos  (bf16 2x mode), issued before the swap chain
                nc.vector.tensor_tensor(dst[:, ssl], raw[:], cos_sb[:, ssl], MULT)
                ps2b = pp.tile([P, CH], F32, tag="ps2", bufs=2)
                nc.tensor.matmul(ps2b[:], psw_sb[:], raw[:], start=True, stop=True)
                tsin = tmp.tile([P, CH], BF16, tag="tsin")
                nc.vector.tensor_tensor(tsin[:], ps2b[:], sin_sb[:, ssl], MULT)
                nc.vector.tensor_add(out=dst[:, ssl], in0=dst[:, ssl], in1=tsin[:])

            def p3_chunk(pair, st, c, hooks):
                # hooks: {jt_index: fn} emitted between jt iterations to
                # interleave next-pair projection bursts into the PE queue
                h0c, h1c = 65 * (2 * pair), 65 * (2 * pair + 1)
                q_sb, k_sb = st["q"], st["k"]
                ssl = slice(c * CH, (c + 1) * CH)
                psA = pp.tile([65, CH], F32, tag="pvA", bufs=1)
                psB = pp.tile([65, CH], F32, tag="pvB", bufs=1)
                njt = 4 * c + 4
                # process jt in pairs: one 64-row-mode burst of 4 score
                # matmuls, then one 128-row-mode burst of 4 PV matmuls —
                # fewer PE tiling-mode switches (each switch drains the
                # array) and more exp() runway for the scalar engine
                for jp in range(njt // 2):
                    pair_jts = (2 * jp, 2 * jp + 1)
                    for jt in pair_jts:
                        for fn in hooks.get(jt, ()):
                            fn()
                    exs = {}
                    for jt in pair_jts:
                        start = max(0, (jt - 4 * c) * P)
                        jsl = slice(jt * P, (jt + 1) * P)
                        isl = slice(c * CH + start, (c + 1) * CH)
                        sc = pp.tile([P, 2, CH], F32, tag="sc", bufs=2)
                        nc.tensor.matmul(
                            sc[:, 0, start:], k_sb[0:DK, jsl], q_sb[0:DK, isl],
                            start=True, stop=True, tile_position=(0, 0))
                        nc.tensor.matmul(
                            sc[:, 1, start:], k_sb[DK:P, jsl], q_sb[DK:P, isl],
                            start=True, stop=True, tile_position=(DK, 0))
                        ex = expp.tile([P, 2, CH], BF16, tag="exp")
                        nc.scalar.activation(
                            ex[:, :, start:], sc[:, :, start:], EXP, scale=0.125)
                        if jt >= 4 * c:
                            # only the [128,128] block at the diagonal needs
                            # the causal mask (0/1 multiply on DVE — faster
                            # than gpsimd affine_select and off its queue)
                            nc.vector.tensor_tensor(
                                ex[:, :, start:start + P],
                                ex[:, :, start:start + P],
                                mask2[:], MULT)
                        exs[jt] = ex
                    for jt in pair_jts:
                        start = max(0, (jt - 4 * c) * P)
                        ex = exs[jt]
                        first, last = (jt == 0), (jt == njt - 1)
                        nc.tensor.matmul(
                            psA[:, start:], v_sb[jt][:, h0c:h0c + 65],
                            ex[:, 0, start:], start=first, stop=last)
                        nc.tensor.matmul(
                            psB[:, start:], v_sb[jt][:, h1c:h1c + 65],
                            ex[:, 1, start:], start=first, stop=last)
                # normalize: denominators sit on partition 64 of psA/psB.
                # Per-head chains staggered across ACT (d0 copy), DVE
                # (recip + final multiply) and GPSIMD (broadcast) so the
                # psA/psB banks free up ~2x sooner for the next chunk's PV.
                for ps_, hoff, nm in ((psA, 0, "A"), (psB, DK, "B")):
                    d0 = tmp.tile([1, CH], F32, tag=f"d0{nm}")
                    nc.scalar.copy(out=d0[:], in_=ps_[DK:DK + 1, :])
                    rcp = tmp.tile([1, CH], F32, tag=f"rcp{nm}")
                    nc.vector.reciprocal_approx_fast(out=rcp[:], in_=d0[:])
                    bc = tmp.tile([DK, CH], F32, tag=f"bc{nm}")
                    nc.gpsimd.partition_broadcast(bc[:], rcp[:], channels=DK)
                    nc.vector.tensor_tensor(
                        att_sb[pair][hoff:hoff + DK, ssl],
                        ps_[0:DK, :], bc[:], MULT)

            wo_box = {}

            def p4_group(ot, c):
                ssl = slice(c * CH, (c + 1) * CH)
                pso = pp.tile([P, CH], F32, tag="ps2", bufs=2)
                for p_ in range(NPAIR):
                    nc.tensor.matmul(
                        pso[:],
                        wo_box["wo"][:, p_, ot * P:(ot + 1) * P],
                        att_sb[p_][:, ssl],
                        start=(p_ == 0), stop=(p_ == NPAIR - 1))
                ob = tmp.tile([P, CH], BF16, tag="ob")
                nc.vector.tensor_copy(out=ob[:], in_=pso[:])
                nc.sync.dma_start(out[ot * P:(ot + 1) * P, ssl], ob[:])

            # phase 1 + prologue interleaved: V-projection st-blocks
            # alternate with pair-0 Q/K projection chunks (each P2 chunk c
            # only needs x columns that the preceding V st-block also needs)
            st_cur = st0
            for c in range(NCH):
                for st in range(4 * c, 4 * c + 4):
                    p1_vproj(st)
                p2_proj(st_cur, c, "q", act_copy=True)
                p2_proj(st_cur, c, "k", act_copy=True)
            p1w_ctx.__exit__(None, None, None)
            for pair in range(NPAIR):
                st_next = p2_prefetch(pair + 1) if pair + 1 < NPAIR else None
                if pair == NPAIR - 2:
                    # prefetch O-projection weights one pair early
                    wo_box["wo"] = work.tile(
                        [P, NPAIR, D], BF16, tag="wo_sb", name="wo_sb", bufs=1)
                    nc.sync.dma_start(wo_box["wo"][:], wo3)
                # last pair: process chunks in order [1,2,3,0] so every
                # chunk's O-projection can interleave into a later chunk
                # (p4 of chunk x hooks into the chunk processed after x)
                corder = (range(NCH) if st_next is not None else (1, 2, 3, 0))
                prev_c = None
                for c in corder:
                    hooks = {}
                    njt = 4 * c + 4
                    if st_next is not None:
                        hooks[njt // 3] = [
                            lambda sn=st_next, cc=c: p2_proj(sn, cc, "q")]
                        hooks[max(njt // 3 + 1, 2 * njt // 3)] = [
                            lambda sn=st_next, cc=c: p2_proj(sn, cc, "k")]
                    elif prev_c is not None:
                        # interleave O-projection of the previously processed
                        # chunk into this chunk
                        npts = min(4, njt - 1)
                        for gi in range(8):
                            key = 1 + (gi % npts) * (njt - 1) // npts
                            hooks.setdefault(key, []).append(
                                lambda o=gi, cc=prev_c: p4_group(o, cc))
                    p3_chunk(pair, st_cur, c, hooks)
                    prev_c = c
                st_cur = st_next
            for ot in range(D // P):
                p4_group(ot, 0)

            pp_ctx.__exit__(None, None, None)

    nc.compile()
    return nc


def _get_nc():
    global _CACHED_NC
    if _CACHED_NC is None:
        _CACHED_NC = build_nc()
    return _CACHED_NC


def make_in_maps(x, token_positions, Wq, Wk, Wv, Wo):
    BF = ml_dtypes.bfloat16
    x = np.asarray(x, dtype=np.float32)
    Wq = np.asarray(Wq, dtype=np.float32)
    Wk = np.asarray(Wk, dtype=np.float32)
    Wv = np.asarray(Wv, dtype=np.float32)
    Wo = np.asarray(Wo, dtype=np.float32)
    pos = np.asarray(token_positions).astype(np.float64)

    freq_idx = np.arange(0, DK, 2, dtype=np.float64)
    inv_freq = 1.0 / (10000.0 ** (freq_idx / DK))
    ang = pos[:, None] * inv_freq[None, :]          # [S, DK/2]
    cos_t = np.cos(ang).astype(np.float32).T        # [DK/2, S]
    sin_t = np.sin(ang).astype(np.float32).T

    pidx = (np.arange(P) % DK) // 2
    cosn = np.ascontiguousarray(cos_t[pidx, :]).astype(BF)   # [128, S]
    sgn = np.where(np.arange(P) % 2 == 0, -1.0, 1.0).astype(np.float32)
    sins = np.ascontiguousarray(sin_t[pidx, :] * sgn[:, None])

    psw = np.zeros((P, P), dtype=np.float32)
    psw[np.arange(P), np.arange(P) ^ 1] = 1.0
    psw = psw.astype(BF)

    in_maps = []
    for core in range(8):
        b, g = core // 2, core % 2
        sl = slice(512 * g, 512 * g + 512)
        in_maps.append({
            "xT": np.ascontiguousarray(x[b].T).astype(BF),
            "wq": np.ascontiguousarray(Wq[sl, :].T).astype(BF),
            "wk": np.ascontiguousarray(Wk[sl, :].T).astype(BF),
            "wv": np.ascontiguousarray(Wv[sl, :].T).astype(BF),
            "wo": np.ascontiguousarray(Wo[:, sl].T).astype(BF),
            "cosn": cosn,
            "sins": sins,
            "psw": psw,
        })
    return in_maps


def kernel(x, token_positions, Wq, Wk, Wv, Wo):
    global LAST_RESULTS
    nc = _get_nc()
    in_maps = make_in_maps(x, token_positions, Wq, Wk, Wv, Wo)
    res = run_bass_kernel_spmd(nc, in_maps, list(range(8)))
    LAST_RESULTS = res
    B = x.shape[0]
    outp = np.empty((B, S, D), dtype=np.float32)
    for b in range(B):
        outp[b] = (res.results[2 * b]["out"].astype(np.float32)
                   + res.results[2 * b + 1]["out"].astype(np.float32)).T
    return outp


# revision 15
# speedup vs baseline: 1.0222x; 1.0222x over previous
"""Causal multi-head attention with RoPE on 8 Trainium2 NeuronCores.

Sharding: core c -> batch b = c // 2, head-group g = c % 2 (8 heads each).
Each core computes q/k/v projections for its 512 output dims, RoPE, causal
attention for its 8 heads, and a partial O-projection. Host sums the two
partial outputs per batch and transposes back.

Device layout notes (v3 — bf16 + resident x + warm PE):
  - x arrives bf16 and stays resident in SBUF ([128, 2048] x 8 k-tiles)
    for the whole kernel: V projection (lhsT slices) and Q/K projections
    (rhs) read it directly — x is DMA'd exactly once.
  - ~20 dummy matmuls at kernel start chew through the initial DMA wait
    so the PE HAM clock-gate is already released (2.4 GHz) when the real
    matmuls begin.
  - All matmul operands are bf16 (PSUM accumulation stays fp32).
  - q/k are kept transposed [d, s] per head-pair tile [128, 2048]
    (head 2p on partitions 0..63, head 2p+1 on 64..127).
  - RoPE: q' = q * cos + swap(q) * sin_signed, where swap is an
    adjacent-partition-pair permutation done with a 128x128 permutation
    matmul; cos (bf16) multiplies the bf16 projection copy at DVE 2x
    rate, sin (f32) multiplies the PSUM swap result.
  - Scores are computed transposed (keys on partitions) so softmax
    needs no DVE reductions: exp() goes straight from PSUM through the
    scalar engine to bf16, the denominator comes from a ones-column
    appended to V, and causal masking is a post-exp affine_select fill
    restricted to the one [128,128] block (both heads in one call) that
    actually straddles the diagonal.
  - v is stored naturally [s, d] with per-head interleaved ones columns
    ([128, 8*65] bf16 tiles) so PV lhsT slices are contiguous.
"""

import numpy as np
import ml_dtypes

import concourse.bass as bass
import concourse.tile as tile
from concourse import bacc, mybir
from concourse.bass_utils import run_bass_kernel_spmd

F32 = mybir.dt.float32
BF16 = mybir.dt.bfloat16
MULT = mybir.AluOpType.mult
IS_GE = mybir.AluOpType.is_ge
EXP = mybir.ActivationFunctionType.Exp

P = 128          # partitions
S = 2048         # sequence length
D = 1024         # model dim
DK = 64          # head dim
HPC = 8          # heads per core
NPAIR = 4        # head pairs per core
KT = 8           # 128-row k-tiles of the contraction dim (D)
CH = 512         # i-chunk width
NCH = S // CH    # 4 i-chunks
NJT = S // P     # 16 j-tiles
NDUMMY = 32      # PE-warming matmuls at kernel start

_CACHED_NC = None
LAST_RESULTS = None


def build_nc():
    nc = bacc.Bacc("TRN2", target_bir_lowering=False, debug=False)

    xT = nc.dram_tensor("xT", [D, S], BF16, kind="ExternalInput").ap()
    wq = nc.dram_tensor("wq", [D, 512], BF16, kind="ExternalInput").ap()
    wk = nc.dram_tensor("wk", [D, 512], BF16, kind="ExternalInput").ap()
    wv = nc.dram_tensor("wv", [D, 512], BF16, kind="ExternalInput").ap()
    wo = nc.dram_tensor("wo", [512, D], BF16, kind="ExternalInput").ap()
    cosn = nc.dram_tensor("cosn", [P, S], BF16, kind="ExternalInput").ap()
    sins = nc.dram_tensor("sins", [P, S], F32, kind="ExternalInput").ap()
    psw = nc.dram_tensor("psw", [P, P], BF16, kind="ExternalInput").ap()
    out = nc.dram_tensor("out", [D, S], BF16, kind="ExternalOutput").ap()

    xT3 = xT.rearrange("(kt p) s -> p kt s", p=P)
    wq3 = wq.rearrange("(kt p) o -> p kt o", p=P)
    wk3 = wk.rearrange("(kt p) o -> p kt o", p=P)
    wv3 = wv.rearrange("(kt p) o -> p kt o", p=P)
    wo3 = wo.rearrange("(pt p) o -> p pt o", p=P)

    with tile.TileContext(nc) as tc:
        with (
            tc.tile_pool(name="persist", bufs=1) as persist,
            tc.tile_pool(name="work", bufs=2) as work,
            tc.tile_pool(name="tmp", bufs=2) as tmp,
            tc.tile_pool(name="expp", bufs=4) as expp,
        ):
            cos_sb = persist.tile([P, S], BF16, tag="cos")
            sin_sb = persist.tile([P, S], F32, tag="sin")
            psw_sb = persist.tile([P, P], BF16, tag="psw")
            xkt = [persist.tile([P, S], BF16, name=f"xkt{kt}", tag=f"xkt{kt}")
                   for kt in range(KT)]
            v_sb = [persist.tile([P, HPC * 65], BF16, name=f"v{jt}", tag=f"v{jt}")
                    for jt in range(NJT)]
            dmy = persist.tile([P, 512], BF16, tag="dmy")
            nc.vector.memset(dmy[:], 0.0)
            # 0/1 causal mask for the [128,128] diagonal block (both heads):
            # mask2[ch, :, i] = 1 if i >= ch else 0
            mask2 = persist.tile([P, 2, P], BF16, tag="mask2")
            nc.vector.memset(mask2[:], 1.0)
            nc.gpsimd.affine_select(
                out=mask2[:], in_=mask2[:], compare_op=IS_GE, fill=0.0,
                base=0, channel_multiplier=-1, pattern=[[0, 2], [1, P]])
            warm = persist.tile([1, 8], F32, tag="warm")
            nc.vector.memset(warm[:], 1.0)
            # touch Exp early so the ~2.7us ACT table load overlaps DMAs
            nc.scalar.activation(warm[:], warm[:], EXP, scale=0.0)
            att_sb = [persist.tile([P, S], BF16, name=f"att{p}", tag=f"att{p}")
                      for p in range(NPAIR)]
            # ones columns of v (strided memset; projection fills the rest)
            for jt in range(NJT):
                v3m = v_sb[jt][:].rearrange("p (h e) -> p h e", e=65)
                nc.vector.memset(v3m[:, :, DK:65], 1.0)

            def p2_prefetch(pair):
                st = {}
                st["q"] = work.tile([P, S], BF16, tag="qpair", name=f"q{pair}")
                st["k"] = work.tile([P, S], BF16, tag="kpair", name=f"k{pair}")
                st["wq"] = work.tile([P, KT, P], BF16, tag="wqp", name=f"wq{pair}", bufs=1)
                st["wk"] = work.tile([P, KT, P], BF16, tag="wkp", name=f"wk{pair}", bufs=1)
                osl = slice(pair * P, (pair + 1) * P)
                nc.sync.dma_start(st["wq"][:], wq3[:, :, osl])
                nc.sync.dma_start(st["wk"][:], wk3[:, :, osl])
                return st

            # ---- single PSUM pool for all phases ----
            pp_ctx = tc.tile_pool(name="pp23", bufs=1, space="PSUM")
            pp = pp_ctx.__enter__()
            p1w_ctx = tc.tile_pool(name="p1w", bufs=1)
            p1w = p1w_ctx.__enter__()

            # PE-warming dummies: no data deps, run during the DMA wait
            for i in range(NDUMMY):
                dps = pp.tile([P, 512], F32, tag="ps2", bufs=2)
                nc.tensor.matmul(dps[:], dmy[:, 0:P], dmy[:],
                                 start=True, stop=True)
            wv_sb = p1w.tile([P, KT, 512], BF16, tag="wv")
            nc.sync.dma_start(wv_sb[:, 0:2, :], wv3[:, 0:2, :])
            for kt in range(KT):
                nc.sync.dma_start(xkt[kt][:, 0:512], xT3[:, kt, 0:512])
            nc.sync.dma_start(wv_sb[:, 2:KT, :], wv3[:, 2:KT, :])
            nc.sync.dma_start(psw_sb[:], psw)
            st0 = p2_prefetch(0)
            for kt in range(KT):
                nc.sync.dma_start(xkt[kt][:, 512:1024], xT3[:, kt, 512:1024])
            nc.sync.dma_start(cos_sb[:], cosn)
            nc.sync.dma_start(sin_sb[:], sins)
            for blk in range(2, 4):
                csl = slice(blk * 512, blk * 512 + 512)
                for kt in range(KT):
                    nc.sync.dma_start(xkt[kt][:, csl], xT3[:, kt, csl])

            def p1_vproj(st):
                ps = pp.tile([P, 512], F32, tag="ps2", bufs=2)
                for kt in range(KT):
                    nc.tensor.matmul(
                        ps[:],
                        xkt[kt][:, st * P:(st + 1) * P],
                        wv_sb[:, kt, :],
                        start=(kt == 0),
                        stop=(kt == KT - 1),
                    )
                v3 = v_sb[st][:].rearrange("p (h e) -> p h e", e=65)
                nc.scalar.copy(
                    out=v3[:, :, 0:DK],
                    in_=ps[:].rearrange("p (h d) -> p h d", d=DK),
                )

            def p2_proj(st, c, which, act_copy=False):
                # one tensor (q or k): 8-matmul projection burst + RoPE
                ssl = slice(c * CH, (c + 1) * CH)
                w_t = st["wq"] if which == "q" else st["wk"]
                dst = st["q"] if which == "q" else st["k"]
                ps2 = pp.tile([P, CH], F32, tag="ps2", bufs=2)
                for kt in range(KT):
                    nc.tensor.matmul(
                        ps2[:], w_t[:, kt, :], xkt[kt][:, ssl],
                        start=(kt == 0), stop=(kt == KT - 1))
                raw = tmp.tile([P, CH], BF16, tag="raw")
                if act_copy:
                    nc.scalar.copy(out=raw[:], in_=ps2[:])
                else:
                    nc.vector.tensor_copy(out=raw[:], in_=ps2[:])
                # dst = raw * cos  (bf16 2x mode), issued before the swap chain
                nc.vector.tensor_tensor(dst[:, ssl], raw[:], cos_sb[:, ssl], MULT)
                ps2b = pp.tile([P, CH], F32, tag="ps2", bufs=2)
                nc.tensor.matmul(ps2b[:], psw_sb[:], raw[:], start=True, stop=True)
                tsin = tmp.tile([P, CH], BF16, tag="tsin")
                nc.vector.tensor_tensor(tsin[:], ps2b[:], sin_sb[:, ssl], MULT)
                nc.vector.tensor_add(out=dst[:, ssl], in0=dst[:, ssl], in1=tsin[:])

            def p3_chunk(pair, st, c, hooks):
                # hooks: {jt_index: fn} emitted between jt iterations to
                # interleave next-pair projection bursts into the PE queue
                h0c, h1c = 65 * (2 * pair), 65 * (2 * pair + 1)
                q_sb, k_sb = st["q"], st["k"]
                ssl = slice(c * CH, (c + 1) * CH)
                psA = pp.tile([65, CH], F32, tag="pvA", bufs=1)
                psB = pp.tile([65, CH], F32, tag="pvB", bufs=1)
                njt = 4 * c + 4
                # process jt in pairs: one 64-row-mode burst of 4 score
                # matmuls, then one 128-row-mode burst of 4 PV matmuls —
                # fewer PE tiling-mode switches (each switch drains the
                # array) and more exp() runway for the scalar engine
                for jp in range(njt // 2):
                    pair_jts = (2 * jp, 2 * jp + 1)
                    for jt in pair_jts:
                        for fn in hooks.get(jt, ()):
                            fn()
                    exs = {}
                    for jt in pair_jts:
                        start = max(0, (jt - 4 * c) * P)
                        jsl = slice(jt * P, (jt + 1) * P)
                        isl = slice(c * CH + start, (c + 1) * CH)
                        sc = pp.tile([P, 2, CH], F32, tag="sc", bufs=2)
                        nc.tensor.matmul(
                            sc[:, 0, start:], k_sb[0:DK, jsl], q_sb[0:DK, isl],
                            start=True, stop=True, tile_position=(0, 0))
                        nc.tensor.matmul(
                            sc[:, 1, start:], k_sb[DK:P, jsl], q_sb[DK:P, isl],
                            start=True, stop=True, tile_position=(DK, 0))
                        ex = expp.tile([P, 2, CH], BF16, tag="exp")
                        nc.scalar.activation(
                            ex[:, :, start:], sc[:, :, start:], EXP, scale=0.125)
                        if jt >= 4 * c:
                            # only the [128,128] block at the diagonal needs
                            # the causal mask (0/1 multiply on DVE — faster
                            # than gpsimd affine_select and off its queue)
                            nc.vector.tensor_tensor(
                                ex[:, :, start:start + P],
                                ex[:, :, start:start + P],
                                mask2[:], MULT)
                        exs[jt] = ex
                    for jt in pair_jts:
                        start = max(0, (jt - 4 * c) * P)
                        ex = exs[jt]
                        first, last = (jt == 0), (jt == njt - 1)
                        nc.tensor.matmul(
                            psA[:, start:], v_sb[jt][:, h0c:h0c + 65],
                            ex[:, 0, start:], start=first, stop=last)
                        nc.tensor.matmul(
                            psB[:, start:], v_sb[jt][:, h1c:h1c + 65],
                            ex[:, 1, start:], start=first, stop=last)
                # normalize: denominators sit on partition 64 of psA/psB.
                # Per-head chains staggered across ACT (d0 copy), DVE
                # (recip + final multiply) and GPSIMD (broadcast) so the
                # psA/psB banks free up ~2x sooner for the next chunk's PV.
                for ps_, hoff, nm in ((psA, 0, "A"), (psB, DK, "B")):
                    d0 = tmp.tile([1, CH], F32, tag=f"d0{nm}")
                    nc.scalar.copy(out=d0[:], in_=ps_[DK:DK + 1, :])
                    rcp = tmp.tile([1, CH], F32, tag=f"rcp{nm}")
                    nc.vector.reciprocal_approx_fast(out=rcp[:], in_=d0[:])
                    bc = tmp.tile([DK, CH], F32, tag=f"bc{nm}")
                    nc.gpsimd.partition_broadcast(bc[:], rcp[:], channels=DK)
                    nc.vector.tensor_tensor(
                        att_sb[pair][hoff:hoff + DK, ssl],
                        ps_[0:DK, :], bc[:], MULT)

            wo_box = {}

            def p4_group(ot, c):
                ssl = slice(c * CH, (c + 1) * CH)
                pso = pp.tile([P, CH], F32, tag="ps2", bufs=2)
                for p_ in range(NPAIR):
                    nc.tensor.matmul(
                        pso[:],
                        wo_box["wo"][:, p_, ot * P:(ot + 1) * P],
                        att_sb[p_][:, ssl],
                        start=(p_ == 0), stop=(p_ == NPAIR - 1))
                ob = tmp.tile([P, CH], BF16, tag="ob")
                nc.vector.tensor_copy(out=ob[:], in_=pso[:])
                nc.sync.dma_start(out[ot * P:(ot + 1) * P, ssl], ob[:])

            # phase 1 + prologue interleaved: V-projection st-blocks
            # alternate with pair-0 Q/K projection chunks (each P2 chunk c
            # only needs x columns that the preceding V st-block also needs)
            st_cur = st0
            for c in range(NCH):
                for st in range(4 * c, 4 * c + 4):
                    p1_vproj(st)
                p2_proj(st_cur, c, "q")
                p2_proj(st_cur, c, "k")
            p1w_ctx.__exit__(None, None, None)
            for pair in range(NPAIR):
                st_next = p2_prefetch(pair + 1) if pair + 1 < NPAIR else None
                if pair == NPAIR - 2:
                    # prefetch O-projection weights one pair early
                    wo_box["wo"] = work.tile(
                        [P, NPAIR, D], BF16, tag="wo_sb", name="wo_sb", bufs=1)
                    nc.sync.dma_start(wo_box["wo"][:], wo3)
                # last pair: process chunks in order [1,2,3,0] so every
                # chunk's O-projection can interleave into a later chunk
                # (p4 of chunk x hooks into the chunk processed after x)
                corder = (range(NCH) if st_next is not None else (1, 2, 3, 0))
                prev_c = None
                for c in corder:
                    hooks = {}
                    njt = 4 * c + 4
                    if st_next is not None:
                        hooks[njt // 3] = [
                            lambda sn=st_next, cc=c: p2_proj(sn, cc, "q")]
                        hooks[max(njt // 3 + 1, 2 * njt // 3)] = [
                            lambda sn=st_next, cc=c: p2_proj(sn, cc, "k")]
                    elif prev_c is not None:
                        # interleave O-projection of the previously processed
                        # chunk into this chunk
                        npts = min(4, njt - 1)
                        for gi in range(8):
                            key = 1 + (gi % npts) * (njt - 1) // npts
                            hooks.setdefault(key, []).append(
                                lambda o=gi, cc=prev_c: p4_group(o, cc))
                    p3_chunk(pair, st_cur, c, hooks)
                    prev_c = c
                st_cur = st_next
            for ot in range(D // P):
                p4_group(ot, 0)

            pp_ctx.__exit__(None, None, None)

    nc.compile()
    return nc


def _get_nc():
    global _CACHED_NC
    if _CACHED_NC is None:
        _CACHED_NC = build_nc()
    return _CACHED_NC


def make_in_maps(x, token_positions, Wq, Wk, Wv, Wo):
    BF = ml_dtypes.bfloat16
    x = np.asarray(x, dtype=np.float32)
    Wq = np.asarray(Wq, dtype=np.float32)
    Wk = np.asarray(Wk, dtype=np.float32)
    Wv = np.asarray(Wv, dtype=np.float32)
    Wo = np.asarray(Wo, dtype=np.float32)
    pos = np.asarray(token_positions).astype(np.float64)

    freq_idx = np.arange(0, DK, 2, dtype=np.float64)
    inv_freq = 1.0 / (10000.0 ** (freq_idx / DK))
    ang = pos[:, None] * inv_freq[None, :]          # [S, DK/2]
    cos_t = np.cos(ang).astype(np.float32).T        # [DK/2, S]
    sin_t = np.sin(ang).astype(np.float32).T

    pidx = (np.arange(P) % DK) // 2
    cosn = np.ascontiguousarray(cos_t[pidx, :]).astype(BF)   # [128, S]
    sgn = np.where(np.arange(P) % 2 == 0, -1.0, 1.0).astype(np.float32)
    sins = np.ascontiguousarray(sin_t[pidx, :] * sgn[:, None])

    psw = np.zeros((P, P), dtype=np.float32)
    psw[np.arange(P), np.arange(P) ^ 1] = 1.0
    psw = psw.astype(BF)

    in_maps = []
    for core in range(8):
        b, g = core // 2, core % 2
        sl = slice(512 * g, 512 * g + 512)
        in_maps.append({
            "xT": np.ascontiguousarray(x[b].T).astype(BF),
            "wq": np.ascontiguousarray(Wq[sl, :].T).astype(BF),
            "wk": np.ascontiguousarray(Wk[sl, :].T).astype(BF),
            "wv": np.ascontiguousarray(Wv[sl, :].T).astype(BF),
            "wo": np.ascontiguousarray(Wo[:, sl].T).astype(BF),
            "cosn": cosn,
            "sins": sins,
            "psw": psw,
        })
    return in_maps


def kernel(x, token_positions, Wq, Wk, Wv, Wo):
    global LAST_RESULTS
    nc = _get_nc()
    in_maps = make_in_maps(x, token_positions, Wq, Wk, Wv, Wo)
    res = run_bass_kernel_spmd(nc, in_maps, list(range(8)))
    LAST_RESULTS = res
    B = x.shape[0]
    outp = np.empty((B, S, D), dtype=np.float32)
    for b in range(B):
        outp[b] = (res.results[2 * b]["out"].astype(np.float32)
                   + res.results[2 * b + 1]["out"].astype(np.float32)).T
    return outp
